# revision 1
# baseline (speedup 1.0000x reference)
"""Trainium2 Bass kernel for nn_Net_91122026151953.

Net (per batch row b):
  xe = x.transpose(0,3,1,2).reshape(B, 240, 180)            # [B,T,180]
  h_enc = lstm_cell_zero_state(xe, Wenc, b)                 # sigmoid/tanh gates, no recurrence
  enc   = softmax(h_enc, axis=-1)
  h_dec = lstm_cell_zero_state(enc, Wdec, b)
  out   = softmax((h_dec.reshape(B,T*180) @ W_out.T + b_out).reshape(B,4,10), -1)

Strategy: pure data-parallel over 8 cores (256 rows each). Row-major
("orientation A") pipeline with bf16 matmuls + intermediates, fp32 final
stage.  All transcendentals use only tanh/exp (sigmoid(x) =
0.5*(1+tanh(x/2)), halves folded into weights/activation scales) so a
single ACT table set is used (exp_and_others), avoiding ~2.7us table
switches.  The decoder bias is folded into Wdec columns (softmax rows sum
to 1).  E^T / h_dec^T for the chained matmuls are produced with DMA xbar
transposes (bf16, 128x128 tiles).
"""

import os
import numpy as np
import ml_dtypes

import concourse.bass as bass
import concourse.tile as tile
from concourse import bacc, mybir
from concourse import bass_utils

BF16 = ml_dtypes.bfloat16
FP32 = np.float32

H = 180          # hidden
T = 240          # timesteps
G3 = 540         # 3 used gates (i, g, o)
NCORES = 8
B_FULL = 2048
BL = B_FULL // NCORES   # rows per core = 256
NB = 32                 # batch rows per block
NBLK = BL // NB         # 8 blocks
LB = 8                  # batch rows per x-load DMA
CPB = NB * T // 128     # chunks (128 rows) per block = 60
MG = 4                  # chunks per macro-group (elementwise batch)
PG = 2                  # chunks per PSUM gates tile
MGB = CPB // MG         # macro-groups per block = 15
TPM = T // MGB          # final-matmul t-steps interleaved per macro-group = 16

AF = mybir.ActivationFunctionType
ALU = mybir.AluOpType
DT = mybir.dt

_PROGRAM = None
LAST_RESULTS = None


def _nsplits(tile_off):
    """Split [tile_off, tile_off+540) at 512-elem (psum bank) boundaries."""
    out = []
    lo = tile_off
    end = tile_off + G3
    while lo < end:
        hi = min(end, ((lo // 512) + 1) * 512)
        out.append((lo, hi - lo))
        lo = hi
    return out


def _build_program():
    nc = bacc.Bacc(None, name="lstm_net")

    xaug = nc.dram_tensor("xaug", [BL, 181, T], DT.bfloat16, kind="ExternalInput")
    wenc = nc.dram_tensor("wenc", [181, G3], DT.bfloat16, kind="ExternalInput")
    wdec = nc.dram_tensor("wdec", [180, G3], DT.bfloat16, kind="ExternalInput")
    w3a = nc.dram_tensor("w3a", [128, T * 40], DT.bfloat16, kind="ExternalInput")
    w3b = nc.dram_tensor("w3b", [52, T * 40], DT.bfloat16, kind="ExternalInput")
    bout = nc.dram_tensor("bout", [40, 1], DT.float32, kind="ExternalInput")
    ident = nc.dram_tensor("ident", [64, 64], DT.float32, kind="ExternalInput")
    out = nc.dram_tensor("out", [BL, 40], DT.float32, kind="ExternalOutput")

    with tile.TileContext(nc) as tc:
        with (
            tc.tile_pool(name="consts", bufs=1) as consts,
            tc.tile_pool(name="xa", bufs=2) as xa_pool,
            tc.tile_pool(name="work", bufs=3) as work,
            tc.tile_pool(name="et", bufs=MG + 2) as et_pool,
            tc.tile_pool(name="hd", bufs=2) as hd_pool,
            tc.tile_pool(name="mini", bufs=2) as mini,
            tc.tile_pool(name="psum", bufs=1, space="PSUM") as psum_pool,
        ):
            # ---- constants ----
            wenc1 = consts.tile([128, G3], DT.bfloat16, tag="wenc1")
            wenc2 = consts.tile([53, G3], DT.bfloat16, tag="wenc2")
            wdec1 = consts.tile([128, G3], DT.bfloat16, tag="wdec1")
            wdec2 = consts.tile([52, G3], DT.bfloat16, tag="wdec2")
            w3a_sb = consts.tile([128, T * 40], DT.bfloat16, tag="w3a")
            w3b_sb = consts.tile([52, T * 40], DT.bfloat16, tag="w3b")
            bout_sb = consts.tile([40, 1], DT.float32, tag="bout")
            ident_sb = consts.tile([64, 64], DT.float32, tag="ident")
            nc.sync.dma_start(out=wenc1[:], in_=wenc[0:128, :])
            nc.sync.dma_start(out=wenc2[:], in_=wenc[128:181, :])
            nc.sync.dma_start(out=wdec1[:], in_=wdec[0:128, :])
            nc.sync.dma_start(out=wdec2[:], in_=wdec[128:180, :])
            nc.sync.dma_start(out=w3a_sb[:], in_=w3a[:, :])
            nc.sync.dma_start(out=w3b_sb[:], in_=w3b[:, :])
            nc.sync.dma_start(out=bout_sb[:], in_=bout[:, :])
            nc.sync.dma_start(out=ident_sb[:], in_=ident[:, :])

            def final_mm_steps(lg, hda_p, hdb_p, t0, t1):
                for t in range(t0, t1):
                    nc.tensor.matmul(
                        lg[:], w3a_sb[:, t * 40:(t + 1) * 40], hda_p[:, :, t],
                        start=(t == 0), stop=False)
                    nc.tensor.matmul(
                        lg[:], w3b_sb[:, t * 40:(t + 1) * 40], hdb_p[0:52, :, t],
                        start=False, stop=(t == T - 1))

            def mini_softmax(lg, blk_prev):
                lgs = mini.tile([40, NB], DT.float32, tag="lgs")
                nc.scalar.copy(lgs[:], lg[:])
                nc.vector.tensor_scalar(
                    lgs[:], lgs[:], bout_sb[:, 0:1], None, ALU.add)
                pst = psum_pool.tile([NB, 40], DT.float32, tag="pst", bufs=1)
                nc.tensor.transpose(pst[:], lgs[:], ident_sb[0:40, 0:40])
                eo = mini.tile([NB, 40], DT.float32, tag="eo")
                nc.scalar.activation(eo[:], pst[:], AF.Exp)
                s4 = mini.tile([NB, 4], DT.float32, tag="s4")
                r4 = mini.tile([NB, 4], DT.float32, tag="r4")
                nc.vector.tensor_reduce(
                    s4[:], eo[:].rearrange("p (g k) -> p g k", k=10),
                    axis=mybir.AxisListType.X, op=ALU.add)
                nc.vector.reciprocal(r4[:], s4[:])
                ob = mini.tile([NB, 40], DT.float32, tag="ob")
                for g in range(4):
                    nc.vector.tensor_scalar(
                        ob[:, g * 10:(g + 1) * 10], eo[:, g * 10:(g + 1) * 10],
                        r4[:, g:g + 1], None, ALU.mult)
                nc.sync.dma_start(
                    out=out[blk_prev * NB:(blk_prev + 1) * NB, :], in_=ob[:])

            prev_hd = None  # (hda, hdb) of previous block
            lg_prev = None

            for blk in range(NBLK):
                # ---- x loads (feature-major slabs, LB batch rows each) ----
                xt1s, xt2s = [], []
                for l in range(NB // LB):
                    b0 = blk * NB + l * LB
                    xt1 = xa_pool.tile([128, LB, T], DT.bfloat16, tag="xt1")
                    xt2 = xa_pool.tile([53, LB, T], DT.bfloat16, tag="xt2")
                    nc.sync.dma_start(
                        out=xt1[:],
                        in_=xaug[b0:b0 + LB, 0:128, :].rearrange("b f t -> f b t"),
                    )
                    nc.sync.dma_start(
                        out=xt2[:],
                        in_=xaug[b0:b0 + LB, 128:181, :].rearrange("b f t -> f b t"),
                    )
                    xt1s.append(xt1)
                    xt2s.append(xt2)

                hda = hd_pool.tile([128, NB, T], DT.bfloat16, tag="hda")
                hdb = hd_pool.tile([128, NB, T], DT.bfloat16, tag="hdb")
                hda_f = hda[:].rearrange("p b t -> p (b t)")
                hdb_f = hdb[:].rearrange("p b t -> p (b t)")

                if prev_hd is not None:
                    lg_prev = psum_pool.tile([40, NB], DT.float32,
                                             tag="logits", bufs=1)

                chunks_per_load = (LB * T) // 128  # 15

                for mg in range(CPB // MG):
                    # interleave previous block's output-linear matmuls so the
                    # PE work spreads across this block instead of bursting
                    if prev_hd is not None:
                        final_mm_steps(lg_prev, prev_hd[0], prev_hd[1],
                                       mg * TPM, (mg + 1) * TPM)
                    tg_e = work.tile([128, MG, G3], DT.bfloat16, tag="tg_e")
                    w2c_e = work.tile([128, MG, H], DT.bfloat16, tag="w2c_e")
                    tc_e = work.tile([128, MG, H], DT.bfloat16, tag="tc_e")
                    h2_e = work.tile([128, MG, H], DT.bfloat16, tag="h2_e")
                    E6 = work.tile([128, MG, 256], DT.bfloat16, tag="E6")
                    s6 = work.tile([128, MG], DT.float32, tag="s6")
                    r6 = work.tile([128, MG], DT.float32, tag="r6")

                    # ---------- encoder matmuls + gate tanh ----------
                    for pgi in range(MG // PG):
                        ps = psum_pool.tile([128, PG * G3], DT.float32, tag="enc_gates", bufs=1)
                        for c in range(PG):
                            cc = mg * MG + pgi * PG + c
                            l, j = divmod(cc, chunks_per_load)
                            lhs1 = xt1s[l][:].rearrange("p b t -> p (b t)")[
                                :, j * 128:(j + 1) * 128]
                            lhs2 = xt2s[l][:].rearrange("p b t -> p (b t)")[
                                :, j * 128:(j + 1) * 128]
                            splits = _nsplits(c * G3)
                            for (n0, nw) in splits:
                                nc.tensor.matmul(
                                    ps[:, n0:n0 + nw], lhs1,
                                    wenc1[:, n0 - c * G3:n0 - c * G3 + nw],
                                    start=True, stop=False)
                            for (n0, nw) in splits:
                                nc.tensor.matmul(
                                    ps[:, n0:n0 + nw], lhs2,
                                    wenc2[:, n0 - c * G3:n0 - c * G3 + nw],
                                    start=False, stop=True)
                        nc.scalar.activation(
                            tg_e[:, pgi * PG:(pgi + 1) * PG, :],
                            ps[:].rearrange("p (c g) -> p c g", g=G3),
                            AF.Tanh)

                    # ---------- encoder cell elementwise ----------
                    i_sl = tg_e[:, :, 0:H]
                    g_sl = tg_e[:, :, H:2 * H]
                    o_sl = tg_e[:, :, 2 * H:3 * H]
                    # w2c = (tanh(i/2)+1)*tanh(g) = 2*c
                    nc.vector.scalar_tensor_tensor(
                        w2c_e[:], i_sl, 1.0, g_sl, ALU.add, ALU.mult)
                    nc.scalar.activation(tc_e[:], w2c_e[:], AF.Tanh, scale=0.5)
                    # h2 = (tanh(o/2)+1)*tanh(c) = 2*h
                    nc.vector.scalar_tensor_tensor(
                        h2_e[:], o_sl, 1.0, tc_e[:], ALU.add, ALU.mult)
                    nc.scalar.activation(E6[:, :, 0:H], h2_e[:], AF.Exp, scale=0.5)
                    nc.vector.tensor_reduce(
                        s6[:], E6[:, :, 0:H], axis=mybir.AxisListType.X, op=ALU.add)
                    nc.vector.reciprocal(r6[:], s6[:])

                    ets = []
                    for c in range(MG):
                        et1 = et_pool.tile([128, 128], DT.bfloat16, tag="et1")
                        et2 = et_pool.tile([128, 128], DT.bfloat16, tag="et2")
                        nc.sync.dma_start_transpose(et1[:], E6[:, c, 0:128])
                        nc.sync.dma_start_transpose(et2[:], E6[:, c, 128:256])
                        ets.append((et1, et2))

                    # ---------- decoder matmuls + gate tanh ----------
                    tg_d = work.tile([128, MG, G3], DT.bfloat16, tag="tg_d")
                    w2c_d = work.tile([128, MG, H], DT.bfloat16, tag="w2c_d")
                    tc_d = work.tile([128, MG, H], DT.bfloat16, tag="tc_d")
                    h2d6 = work.tile([128, MG, 256], DT.bfloat16, tag="h2d6")
                    for pgi in range(MG // PG):
                        psd = psum_pool.tile([128, PG * G3], DT.float32, tag="dec_gates", bufs=1)
                        for c in range(PG):
                            et1, et2 = ets[pgi * PG + c]
                            splits = _nsplits(c * G3)
                            for (n0, nw) in splits:
                                nc.tensor.matmul(
                                    psd[:, n0:n0 + nw], et1[:],
                                    wdec1[:, n0 - c * G3:n0 - c * G3 + nw],
                                    start=True, stop=False)
                            for (n0, nw) in splits:
                                nc.tensor.matmul(
                                    psd[:, n0:n0 + nw], et2[0:52, :],
                                    wdec2[:, n0 - c * G3:n0 - c * G3 + nw],
                                    start=False, stop=True)
                        for c in range(PG):
                            nc.scalar.activation(
                                tg_d[:, pgi * PG + c, :],
                                psd[:, c * G3:(c + 1) * G3],
                                AF.Tanh,
                                scale=r6[:, pgi * PG + c:pgi * PG + c + 1])

                    # ---------- decoder cell elementwise ----------
                    i_d = tg_d[:, :, 0:H]
                    g_d = tg_d[:, :, H:2 * H]
                    o_d = tg_d[:, :, 2 * H:3 * H]
                    nc.vector.scalar_tensor_tensor(
                        w2c_d[:], i_d, 1.0, g_d, ALU.add, ALU.mult)
                    nc.scalar.activation(tc_d[:], w2c_d[:], AF.Tanh, scale=0.5)
                    nc.vector.scalar_tensor_tensor(
                        h2d6[:, :, 0:H], o_d, 1.0, tc_d[:], ALU.add, ALU.mult)

                    for c in range(MG):
                        cc = mg * MG + c
                        nc.sync.dma_start_transpose(
                            hda_f[:, cc * 128:(cc + 1) * 128], h2d6[:, c, 0:128])
                        nc.sync.dma_start_transpose(
                            hdb_f[:, cc * 128:(cc + 1) * 128], h2d6[:, c, 128:256])

                # end of macro-group loop: previous block's logits are done
                if prev_hd is not None:
                    mini_softmax(lg_prev, blk - 1)
                prev_hd = (hda, hdb)

            # tail: last block's output linear + softmax
            lg_prev = psum_pool.tile([40, NB], DT.float32, tag="logits", bufs=1)
            final_mm_steps(lg_prev, prev_hd[0], prev_hd[1], 0, T)
            mini_softmax(lg_prev, NBLK - 1)

    nc.finalize()
    return nc


def _get_program():
    global _PROGRAM
    if _PROGRAM is None:
        _PROGRAM = _build_program()
    return _PROGRAM


def _prep_lstm_weights(Wih, bih, bhh):
    W = np.asarray(Wih, np.float32)
    b = np.asarray(bih, np.float32) + np.asarray(bhh, np.float32)
    # torch gate order i, f, g, o; f unused (zero state). Halve i/o for
    # the tanh half-angle sigmoid identity.
    Wp = np.concatenate([0.5 * W[0:H], W[2 * H:3 * H], 0.5 * W[3 * H:4 * H]], 0)
    bp = np.concatenate([0.5 * b[0:H], b[2 * H:3 * H], 0.5 * b[3 * H:4 * H]], 0)
    return Wp, bp  # [540, 180], [540]


def kernel(x, W_ih_enc, b_ih_enc, b_hh_enc, W_ih_dec, b_ih_dec, b_hh_dec,
           W_out, b_out):
    global LAST_RESULTS
    x = np.asarray(x)
    B = x.shape[0]
    assert B == B_FULL, f"kernel hardcoded for B={B_FULL}, got {B}"

    # x[b, c, s, t] with feature f = c*60+s -> xaug[b, f, t]; row of ones
    # provides the encoder bias via the augmented contraction dim.
    xaug = np.empty((B, 181, T), BF16)
    xaug[:, :180, :] = x.reshape(B, 180, T)
    xaug[:, 180, :] = 1.0

    We, be = _prep_lstm_weights(W_ih_enc, b_ih_enc, b_hh_enc)
    wenc = np.concatenate([We.T, be[None, :]], 0).astype(BF16)  # [181, 540]

    Wd, bd = _prep_lstm_weights(W_ih_dec, b_ih_dec, b_hh_dec)
    # softmax rows sum to 1 -> bias folds into every column of Wdec
    wdec = (Wd.T + bd[None, :]).astype(BF16)  # [180, 540]

    # logits use h = h2/2 -> fold the 0.5 into W_out; W3[h, t, j]
    W3 = (0.5 * np.asarray(W_out, np.float32)).reshape(40, T, H)
    W3 = np.ascontiguousarray(W3.transpose(2, 1, 0))  # [180, 240, 40]
    w3a = np.ascontiguousarray(W3[0:128]).reshape(128, T * 40).astype(BF16)
    w3b = np.ascontiguousarray(W3[128:180]).reshape(52, T * 40).astype(BF16)

    bout = np.asarray(b_out, np.float32).reshape(40, 1)
    ident = np.eye(64, dtype=np.float32)

    nc = _get_program()
    in_maps = []
    for c in range(NCORES):
        in_maps.append({
            "xaug": xaug[c * BL:(c + 1) * BL],
            "wenc": wenc,
            "wdec": wdec,
            "w3a": w3a,
            "w3b": w3b,
            "bout": bout,
            "ident": ident,
        })
    trace = bool(int(os.environ.get("KERNEL_TRACE", "0")))
    res = bass_utils.run_bass_kernel_spmd(
        nc, in_maps, core_ids=list(range(NCORES)), trace=trace)
    LAST_RESULTS = res
    out = np.concatenate([r["out"] for r in res.results], 0)  # [B, 40]
    return out.reshape(B, 4, 10).astype(np.float32)



# revision 8
# speedup vs baseline: 1.2591x; 1.2591x over previous
"""Trainium2 Bass kernel for nn_Net_91122026151953.

Net (per batch row b):
  xe = x.transpose(0,3,1,2).reshape(B, 240, 180)            # [B,T,180]
  h_enc = lstm_cell_zero_state(xe, Wenc, b)                 # sigmoid/tanh gates, no recurrence
  enc   = softmax(h_enc, axis=-1)
  h_dec = lstm_cell_zero_state(enc, Wdec, b)
  out   = softmax((h_dec.reshape(B,T*180) @ W_out.T + b_out).reshape(B,4,10), -1)

Strategy: pure data-parallel over 8 cores (256 rows each).  "Orientation
B": features/hidden on SBUF partitions, (t, b) on the free dim with all
256 batch rows contiguous (t-outer).  This removes every transpose (the
encoder input arrives feature-major from a host-side transpose; the
decoder input E^T and the final-linear input h_dec^T are produced
directly in the needed layout), and lets the output Linear run as 4
N=256 matmuls per chunk accumulating into one persistent [40,256] PSUM
tile instead of thousands of N=32 matmuls.

All transcendentals use only tanh/exp (sigmoid(x) = 0.5*(1+tanh(x/2)),
halves folded into weights) so a single ACT table set is used.  The
decoder bias is folded into Wdec columns (softmax rows sum to 1); the
encoder softmax normalizer is applied to E explicitly (PE ones-matmul
column sum -> reciprocal -> PE broadcast -> multiply).
"""

import os
import numpy as np
import ml_dtypes

import concourse.bass as bass
import concourse.tile as tile
from concourse import bacc, mybir
from concourse import bass_utils

BF16 = ml_dtypes.bfloat16
FP32 = np.float32

H = 180           # hidden
T = 240           # timesteps
G3 = 540          # 3 used gates (i, g, o)
NCORES = 8
B_FULL = 2048
BL = B_FULL // NCORES   # rows per core = 256
CW = 512                # chunk width (free columns) = 2 t-steps * 256 b
TPC = CW // BL          # t-steps per chunk = 2
NCHUNK = T // TPC       # 120 chunks
SL = 4                  # chunks per x-load slab

# gate column layout in the permuted 540-wide weight matrices
#   iA: 0:128    gA: 128:256   oA: 256:384   (h = 0..127)
#   iB: 384:436  gB: 436:488   oB: 488:540   (h = 128..179)
GRPS = [(0, 128), (128, 128), (256, 128), (384, 52), (436, 52), (488, 52)]
HB = 52           # second h block size

AF = mybir.ActivationFunctionType
ALU = mybir.AluOpType
DT = mybir.dt

_PROGRAM = None
LAST_RESULTS = None


def _build_program():
    nc = bacc.Bacc(None, name="lstm_net_b")

    xt = nc.dram_tensor("xt", [181, T, BL], DT.bfloat16, kind="ExternalInput")
    wenc = nc.dram_tensor("wenc", [181, G3], DT.bfloat16, kind="ExternalInput")
    wdec = nc.dram_tensor("wdec", [180, G3], DT.bfloat16, kind="ExternalInput")
    w3a = nc.dram_tensor("w3a", [128, T * 40], DT.bfloat16, kind="ExternalInput")
    w3b = nc.dram_tensor("w3b", [HB, T * 40], DT.bfloat16, kind="ExternalInput")
    bout = nc.dram_tensor("bout", [40, 1], DT.float32, kind="ExternalInput")
    ident = nc.dram_tensor("ident", [64, 64], DT.float32, kind="ExternalInput")
    onescol = nc.dram_tensor("onescol", [128, 1], DT.bfloat16, kind="ExternalInput")
    onesrow = nc.dram_tensor("onesrow", [1, 128], DT.bfloat16, kind="ExternalInput")
    blk40 = nc.dram_tensor("blk40", [40, 4], DT.float32, kind="ExternalInput")
    blk4 = nc.dram_tensor("blk4", [4, 40], DT.float32, kind="ExternalInput")
    out = nc.dram_tensor("out", [BL, 40], DT.float32, kind="ExternalOutput")

    with tile.TileContext(nc) as tc:
        with (
            tc.tile_pool(name="consts", bufs=1) as consts,
            tc.tile_pool(name="xa", bufs=2) as xa_pool,
            tc.tile_pool(name="gt", bufs=2) as gt_pool,       # gate tanh outputs
            tc.tile_pool(name="mid", bufs=2) as mid_pool,     # w2c/tc/h2/E
            tc.tile_pool(name="en", bufs=2) as en_pool,       # normalized E
            tc.tile_pool(name="hd", bufs=2) as hd_pool,       # decoder h2
            tc.tile_pool(name="sm", bufs=2) as sm_pool,       # small softmax bits
            tc.tile_pool(name="fin", bufs=1) as fin_pool,
            tc.tile_pool(name="psum", bufs=1, space="PSUM") as psum_pool,
        ):
            # ---- constants ----
            wencA = consts.tile([128, G3], DT.bfloat16, tag="wencA")
            wencB = consts.tile([53, G3], DT.bfloat16, tag="wencB")
            wdecA = consts.tile([128, G3], DT.bfloat16, tag="wdecA")
            wdecB = consts.tile([HB, G3], DT.bfloat16, tag="wdecB")
            w3a_sb = consts.tile([128, T * 40], DT.bfloat16, tag="w3a")
            w3b_sb = consts.tile([HB, T * 40], DT.bfloat16, tag="w3b")
            bout_sb = consts.tile([40, 1], DT.float32, tag="bout")
            ident_sb = consts.tile([64, 64], DT.float32, tag="ident")
            onescol_sb = consts.tile([128, 1], DT.bfloat16, tag="onescol")
            onesrow_sb = consts.tile([1, 128], DT.bfloat16, tag="onesrow")
            blk40_sb = consts.tile([40, 4], DT.float32, tag="blk40")
            blk4_sb = consts.tile([4, 40], DT.float32, tag="blk4")
            nc.sync.dma_start(out=wencA[:], in_=wenc[0:128, :])
            nc.sync.dma_start(out=wencB[:], in_=wenc[128:181, :])
            nc.sync.dma_start(out=wdecA[:], in_=wdec[0:128, :])
            nc.sync.dma_start(out=wdecB[:], in_=wdec[128:180, :])
            nc.sync.dma_start(out=w3a_sb[:], in_=w3a[:, :])
            nc.sync.dma_start(out=w3b_sb[:], in_=w3b[:, :])
            nc.sync.dma_start(out=bout_sb[:], in_=bout[:, :])
            nc.sync.dma_start(out=ident_sb[:], in_=ident[:, :])
            nc.sync.dma_start(out=onescol_sb[:], in_=onescol[:, :])
            nc.sync.dma_start(out=onesrow_sb[:], in_=onesrow[:, :])
            nc.sync.dma_start(out=blk40_sb[:], in_=blk40[:, :])
            nc.sync.dma_start(out=blk4_sb[:], in_=blk4[:, :])

            xt_flat = xt[:, :, :].rearrange("f t b -> f (t b)")

            # persistent logits accumulator (one PSUM bank, whole kernel)
            acc = psum_pool.tile([40, BL], DT.float32, tag="acc", bufs=1)

            xslabs = [None] * (NCHUNK // SL)

            for c in range(NCHUNK):
                # ---- x slab load (SL chunks at a time) ----
                if c % SL == 0:
                    si = c // SL
                    xtA = xa_pool.tile([128, SL * CW], DT.bfloat16, tag="xtA")
                    xtB = xa_pool.tile([53, SL * CW], DT.bfloat16, tag="xtB")
                    nc.sync.dma_start(
                        out=xtA[:], in_=xt_flat[0:128, c * CW:(c + SL) * CW])
                    nc.sync.dma_start(
                        out=xtB[:], in_=xt_flat[128:181, c * CW:(c + SL) * CW])
                    xslabs[si] = (xtA, xtB)
                xtA, xtB = xslabs[c // SL]
                co = (c % SL) * CW
                rhsA = xtA[:, co:co + CW]
                rhsB = xtB[:, co:co + CW]

                # ---- encoder gates: 6 matmul groups + tanh ----
                egt = []   # tanh(gates) tiles: [iA,gA,oA] 128p, [iB,gB,oB] 52p
                for gi, (g0, gw) in enumerate(GRPS):
                    tag = "egA" if gw == 128 else "gB"
                    ps = psum_pool.tile([gw, CW], DT.float32, tag=tag, bufs=2)
                    nc.tensor.matmul(ps[:], wencA[:, g0:g0 + gw], rhsA,
                                     start=True, stop=False)
                    nc.tensor.matmul(ps[:], wencB[:, g0:g0 + gw], rhsB,
                                     start=False, stop=True)
                    tg = gt_pool.tile([gw, CW], DT.bfloat16, tag=f"etg{gi}")
                    nc.scalar.activation(tg[:], ps[:], AF.Tanh)
                    egt.append(tg)

                # ---- encoder cell elementwise ----
                # w2c = (tanh(i/2)+1)*tanh(g) = 2*c ; tc = tanh(c)
                # h2  = (tanh(o/2)+1)*tc = 2*h ; E = exp(h2/2) = exp(h)
                w2cA = mid_pool.tile([128, CW], DT.bfloat16, tag="w2cA")
                w2cB = mid_pool.tile([HB, CW], DT.bfloat16, tag="w2cB")
                nc.vector.scalar_tensor_tensor(
                    w2cA[:], egt[0][:], 1.0, egt[1][:], ALU.add, ALU.mult)
                nc.vector.scalar_tensor_tensor(
                    w2cB[:], egt[3][:], 1.0, egt[4][:], ALU.add, ALU.mult)
                tcA = mid_pool.tile([128, CW], DT.bfloat16, tag="tcA")
                tcB = mid_pool.tile([HB, CW], DT.bfloat16, tag="tcB")
                nc.scalar.activation(tcA[:], w2cA[:], AF.Tanh, scale=0.5)
                nc.scalar.activation(tcB[:], w2cB[:], AF.Tanh, scale=0.5)
                h2A = mid_pool.tile([128, CW], DT.bfloat16, tag="h2A")
                h2B = mid_pool.tile([HB, CW], DT.bfloat16, tag="h2B")
                nc.vector.scalar_tensor_tensor(
                    h2A[:], egt[2][:], 1.0, tcA[:], ALU.add, ALU.mult)
                nc.vector.scalar_tensor_tensor(
                    h2B[:], egt[5][:], 1.0, tcB[:], ALU.add, ALU.mult)
                EA = mid_pool.tile([128, CW], DT.bfloat16, tag="EA")
                EB = mid_pool.tile([HB, CW], DT.bfloat16, tag="EB")
                nc.scalar.activation(EA[:], h2A[:], AF.Exp, scale=0.5)
                nc.scalar.activation(EB[:], h2B[:], AF.Exp, scale=0.5)

                # ---- softmax normalizer: colsum -> recip -> broadcast ----
                smp = psum_pool.tile([128, CW], DT.float32, tag="smp", bufs=1)
                nc.tensor.matmul(smp[0:1, :], onescol_sb[:, 0:1], EA[:],
                                 start=True, stop=False)
                nc.tensor.matmul(smp[0:1, :], onescol_sb[0:HB, 0:1], EB[:],
                                 start=False, stop=True)
                rb = sm_pool.tile([1, CW], DT.bfloat16, tag="rb")
                with nc.allow_low_precision(reason="softmax recip in bf16"):
                    nc.vector.reciprocal(rb[:], smp[0:1, :])
                nc.tensor.matmul(smp[:, :], onesrow_sb[0:1, :], rb[:],
                                 start=True, stop=True)
                rbc = sm_pool.tile([128, CW], DT.bfloat16, tag="rbc")
                nc.vector.tensor_copy(rbc[:], smp[:, :])
                EnA = en_pool.tile([128, CW], DT.bfloat16, tag="EnA")
                EnB = en_pool.tile([HB, CW], DT.bfloat16, tag="EnB")
                nc.vector.tensor_mul(EnA[:], EA[:], rbc[:])
                nc.vector.tensor_mul(EnB[:], EB[:], rbc[0:HB, :])

                # ---- decoder gates: 6 matmul groups + tanh ----
                dgt = []
                for gi, (g0, gw) in enumerate(GRPS):
                    tag = "dgA" if gw == 128 else "gB"
                    ps = psum_pool.tile([gw, CW], DT.float32, tag=tag, bufs=2)
                    nc.tensor.matmul(ps[:], wdecA[:, g0:g0 + gw], EnA[:],
                                     start=True, stop=False)
                    nc.tensor.matmul(ps[:], wdecB[:, g0:g0 + gw], EnB[:],
                                     start=False, stop=True)
                    tg = gt_pool.tile([gw, CW], DT.bfloat16, tag=f"dtg{gi}")
                    nc.scalar.activation(tg[:], ps[:], AF.Tanh)
                    dgt.append(tg)

                # ---- decoder cell elementwise ----
                wdA = mid_pool.tile([128, CW], DT.bfloat16, tag="wdA")
                wdB = mid_pool.tile([HB, CW], DT.bfloat16, tag="wdB")
                nc.vector.scalar_tensor_tensor(
                    wdA[:], dgt[0][:], 1.0, dgt[1][:], ALU.add, ALU.mult)
                nc.vector.scalar_tensor_tensor(
                    wdB[:], dgt[3][:], 1.0, dgt[4][:], ALU.add, ALU.mult)
                tdA = mid_pool.tile([128, CW], DT.bfloat16, tag="tdA")
                tdB = mid_pool.tile([HB, CW], DT.bfloat16, tag="tdB")
                nc.scalar.activation(tdA[:], wdA[:], AF.Tanh, scale=0.5)
                nc.scalar.activation(tdB[:], wdB[:], AF.Tanh, scale=0.5)
                # h2_dec; the 0.5 to get h is folded into W_out
                hdA = hd_pool.tile([128, CW], DT.bfloat16, tag="hdA")
                hdB = hd_pool.tile([HB, CW], DT.bfloat16, tag="hdB")
                nc.vector.scalar_tensor_tensor(
                    hdA[:], dgt[2][:], 1.0, tdA[:], ALU.add, ALU.mult)
                nc.vector.scalar_tensor_tensor(
                    hdB[:], dgt[5][:], 1.0, tdB[:], ALU.add, ALU.mult)

                # ---- output linear accumulation ----
                for tt in range(TPC):
                    t = c * TPC + tt
                    first = (t == 0)
                    last = (t == T - 1)
                    nc.tensor.matmul(
                        acc[:], w3a_sb[:, t * 40:(t + 1) * 40],
                        hdA[:, tt * BL:(tt + 1) * BL],
                        start=first, stop=False)
                    nc.tensor.matmul(
                        acc[:], w3b_sb[:, t * 40:(t + 1) * 40],
                        hdB[:, tt * BL:(tt + 1) * BL],
                        start=False, stop=last)

            # ---- end stage: bias, 4x10 group softmax, transpose, store ----
            lg = fin_pool.tile([40, BL], DT.float32, tag="lg")
            nc.vector.tensor_scalar(lg[:], acc[:], bout_sb[:, 0:1], None,
                                    ALU.add)
            eo = fin_pool.tile([40, BL], DT.float32, tag="eo")
            nc.scalar.activation(eo[:], lg[:], AF.Exp)
            ep = psum_pool.tile([128, CW], DT.float32, tag="smp", bufs=1)
            # group sums: [4, 256] = blk40^T(40x4) @ eo  (fp32 matmul)
            nc.tensor.matmul(ep[0:4, 0:BL], blk40_sb[:], eo[:],
                             start=True, stop=True)
            r4 = fin_pool.tile([4, BL], DT.float32, tag="r4")
            nc.vector.reciprocal(r4[:], ep[0:4, 0:BL])
            # broadcast r4 back to 40 partitions: blk4^T(4x40) @ r4
            nc.tensor.matmul(ep[0:40, BL:2 * BL], blk4_sb[:], r4[:],
                             start=True, stop=True)
            ob = fin_pool.tile([40, BL], DT.float32, tag="ob")
            nc.vector.tensor_tensor(ob[:], eo[:], ep[0:40, BL:2 * BL],
                                    ALU.mult)
            # transpose [40, 256] -> [256, 40] in two PE transposes,
            # reusing the smp psum bank (all prior reads complete by then)
            nc.tensor.transpose(ep[:, 0:40], ob[:, 0:128],
                                ident_sb[0:40, 0:40])
            nc.tensor.transpose(ep[:, 40:80], ob[:, 128:256],
                                ident_sb[0:40, 0:40])
            ot1 = fin_pool.tile([128, 40], DT.float32, tag="ot1")
            ot2 = fin_pool.tile([128, 40], DT.float32, tag="ot2")
            nc.scalar.copy(ot1[:], ep[:, 0:40])
            nc.scalar.copy(ot2[:], ep[:, 40:80])
            nc.sync.dma_start(out=out[0:128, :], in_=ot1[:])
            nc.sync.dma_start(out=out[128:256, :], in_=ot2[:])

    nc.finalize()
    return nc


def _get_program():
    global _PROGRAM
    if _PROGRAM is None:
        _PROGRAM = _build_program()
    return _PROGRAM


def _prep_lstm_weights(Wih, bih, bhh):
    W = np.asarray(Wih, np.float32)
    b = np.asarray(bih, np.float32) + np.asarray(bhh, np.float32)
    # torch gate order i, f, g, o; f unused (zero state). Halve i/o for
    # the tanh half-angle sigmoid identity.
    Wp = np.concatenate([0.5 * W[0:H], W[2 * H:3 * H], 0.5 * W[3 * H:4 * H]], 0)
    bp = np.concatenate([0.5 * b[0:H], b[2 * H:3 * H], 0.5 * b[3 * H:4 * H]], 0)
    return Wp, bp  # [540, 180], [540]


# permutation of the 540 (i,g,o)-rows into the on-chip column layout
_PERM = np.concatenate([
    np.arange(0, 128),          # iA
    np.arange(180, 308),        # gA
    np.arange(360, 488),        # oA
    np.arange(128, 180),        # iB
    np.arange(308, 360),        # gB
    np.arange(488, 540),        # oB
])


def kernel(x, W_ih_enc, b_ih_enc, b_hh_enc, W_ih_dec, b_ih_dec, b_hh_dec,
           W_out, b_out):
    global LAST_RESULTS
    x = np.asarray(x)
    B = x.shape[0]
    assert B == B_FULL, f"kernel hardcoded for B={B_FULL}, got {B}"

    # x[b, c, s, t] with feature f = c*60+s -> per-core xt[f, t, b];
    # row f=180 of ones provides the encoder bias via the augmented
    # contraction dim.
    xr = x.reshape(B, H, T)
    xts = []
    for c in range(NCORES):
        xt = np.empty((181, T, BL), BF16)
        xt[:180] = xr[c * BL:(c + 1) * BL].transpose(1, 2, 0)
        xt[180] = 1.0
        xts.append(xt)

    We, be = _prep_lstm_weights(W_ih_enc, b_ih_enc, b_hh_enc)
    wenc = np.concatenate([We.T, be[None, :]], 0)[:, _PERM]
    wenc = np.ascontiguousarray(wenc).astype(BF16)  # [181, 540]

    Wd, bd = _prep_lstm_weights(W_ih_dec, b_ih_dec, b_hh_dec)
    # softmax rows sum to 1 -> bias folds into every column of Wdec
    wdec = (Wd.T + bd[None, :])[:, _PERM]
    wdec = np.ascontiguousarray(wdec).astype(BF16)  # [180, 540]

    # logits use h = h2/2 -> fold the 0.5 into W_out; W3[h, t, j]
    W3 = (0.5 * np.asarray(W_out, np.float32)).reshape(40, T, H)
    W3 = np.ascontiguousarray(W3.transpose(2, 1, 0))  # [180, 240, 40]
    w3a = np.ascontiguousarray(W3[0:128]).reshape(128, T * 40).astype(BF16)
    w3b = np.ascontiguousarray(W3[128:180]).reshape(HB, T * 40).astype(BF16)

    bout = np.asarray(b_out, np.float32).reshape(40, 1)
    ident = np.eye(64, dtype=np.float32)
    onescol = np.ones((128, 1), BF16)
    onesrow = np.ones((1, 128), BF16)
    gidx = np.arange(40) // 10
    blk40 = (gidx[:, None] == np.arange(4)[None, :]).astype(np.float32)
    blk4 = np.ascontiguousarray(blk40.T)

    nc = _get_program()
    in_maps = []
    for c in range(NCORES):
        in_maps.append({
            "xt": xts[c],
            "wenc": wenc,
            "wdec": wdec,
            "w3a": w3a,
            "w3b": w3b,
            "bout": bout,
            "ident": ident,
            "onescol": onescol,
            "onesrow": onesrow,
            "blk40": blk40,
            "blk4": blk4,
        })
    trace = bool(int(os.environ.get("KERNEL_TRACE", "0")))
    res = bass_utils.run_bass_kernel_spmd(
        nc, in_maps, core_ids=list(range(NCORES)), trace=trace)
    LAST_RESULTS = res
    out = np.concatenate([r["out"] for r in res.results], 0)  # [B, 40]
    return out.reshape(B, 4, 10).astype(np.float32)


# revision 9
# speedup vs baseline: 1.7332x; 1.3765x over previous
"""Trainium2 Bass kernel for nn_Net_91122026151953.

Net (per batch row b):
  xe = x.transpose(0,3,1,2).reshape(B, 240, 180)            # [B,T,180]
  h_enc = lstm_cell_zero_state(xe, Wenc, b)                 # sigmoid/tanh gates, no recurrence
  enc   = softmax(h_enc, axis=-1)
  h_dec = lstm_cell_zero_state(enc, Wdec, b)
  out   = softmax((h_dec.reshape(B,T*180) @ W_out.T + b_out).reshape(B,4,10), -1)

Strategy: pure data-parallel over 8 cores (256 rows each).  "Orientation
B": features/hidden on SBUF partitions, (t, b) on the free dim with all
256 batch rows contiguous (t-outer).  This removes every transpose (the
encoder input arrives feature-major from a host-side transpose; the
decoder input E^T and the final-linear input h_dec^T are produced
directly in the needed layout), and lets the output Linear run as 4
N=256 matmuls per chunk accumulating into one persistent [40,256] PSUM
tile instead of thousands of N=32 matmuls.

All transcendentals use only tanh/exp (sigmoid(x) = 0.5*(1+tanh(x/2)),
halves folded into weights) so a single ACT table set is used.  The
decoder bias is folded into Wdec columns (softmax rows sum to 1); the
encoder softmax normalizer is applied to E explicitly (PE ones-matmul
column sum -> reciprocal -> PE broadcast -> multiply).
"""

import os
import numpy as np
import ml_dtypes

import concourse.bass as bass
import concourse.tile as tile
from concourse import bacc, mybir
from concourse import bass_utils

BF16 = ml_dtypes.bfloat16
FP32 = np.float32

H = 180           # hidden
T = 240           # timesteps
G3 = 540          # 3 used gates (i, g, o)
NCORES = 8
B_FULL = 2048
BL = B_FULL // NCORES   # rows per core = 256
CW = 512                # chunk width (free columns) = 2 t-steps * 256 b
TPC = CW // BL          # t-steps per chunk = 2
NCHUNK = T // TPC       # 120 chunks
SL = 4                  # chunks per x-load slab

# gate column layout in the permuted 540-wide weight matrices
#   iA: 0:128    gA: 128:256   oA: 256:384   (h = 0..127)
#   iB: 384:436  gB: 436:488   oB: 488:540   (h = 128..179)
GRPS = [(0, 128), (128, 128), (256, 128), (384, 52), (436, 52), (488, 52)]
HB = 52           # second h block size

AF = mybir.ActivationFunctionType
ALU = mybir.AluOpType
DT = mybir.dt

_PROGRAM = None
LAST_RESULTS = None


def _build_program():
    nc = bacc.Bacc(None, name="lstm_net_b")

    xt = nc.dram_tensor("xt", [181, T, BL], DT.bfloat16, kind="ExternalInput")
    wenc = nc.dram_tensor("wenc", [181, G3], DT.bfloat16, kind="ExternalInput")
    wdec = nc.dram_tensor("wdec", [180, G3], DT.bfloat16, kind="ExternalInput")
    w3a = nc.dram_tensor("w3a", [128, T * 40], DT.bfloat16, kind="ExternalInput")
    w3b = nc.dram_tensor("w3b", [HB, T * 40], DT.bfloat16, kind="ExternalInput")
    bout = nc.dram_tensor("bout", [40, 1], DT.float32, kind="ExternalInput")
    ident = nc.dram_tensor("ident", [64, 64], DT.float32, kind="ExternalInput")
    onescol = nc.dram_tensor("onescol", [128, 1], DT.bfloat16, kind="ExternalInput")
    onesrow = nc.dram_tensor("onesrow", [1, 128], DT.bfloat16, kind="ExternalInput")
    blk40 = nc.dram_tensor("blk40", [40, 4], DT.float32, kind="ExternalInput")
    blk4 = nc.dram_tensor("blk4", [4, 40], DT.float32, kind="ExternalInput")
    out = nc.dram_tensor("out", [BL, 40], DT.float32, kind="ExternalOutput")

    with tile.TileContext(nc) as tc:
        with (
            tc.tile_pool(name="consts", bufs=1) as consts,
            tc.tile_pool(name="xa", bufs=2) as xa_pool,
            tc.tile_pool(name="gt", bufs=2) as gt_pool,       # gate tanh outputs
            tc.tile_pool(name="mid", bufs=2) as mid_pool,     # w2c/tc/h2/E
            tc.tile_pool(name="en", bufs=2) as en_pool,       # normalized E
            tc.tile_pool(name="hd", bufs=2) as hd_pool,       # decoder h2
            tc.tile_pool(name="sm", bufs=2) as sm_pool,       # small softmax bits
            tc.tile_pool(name="fin", bufs=1) as fin_pool,
            tc.tile_pool(name="psum", bufs=1, space="PSUM") as psum_pool,
        ):
            # ---- constants ----
            wencA = consts.tile([128, G3], DT.bfloat16, tag="wencA")
            wencB = consts.tile([53, G3], DT.bfloat16, tag="wencB")
            wdecA = consts.tile([128, G3], DT.bfloat16, tag="wdecA")
            wdecB = consts.tile([HB, G3], DT.bfloat16, tag="wdecB")
            w3a_sb = consts.tile([128, T * 40], DT.bfloat16, tag="w3a")
            w3b_sb = consts.tile([HB, T * 40], DT.bfloat16, tag="w3b")
            bout_sb = consts.tile([40, 1], DT.float32, tag="bout")
            ident_sb = consts.tile([64, 64], DT.float32, tag="ident")
            onescol_sb = consts.tile([128, 1], DT.bfloat16, tag="onescol")
            onesrow_sb = consts.tile([1, 128], DT.bfloat16, tag="onesrow")
            blk40_sb = consts.tile([40, 4], DT.float32, tag="blk40")
            blk4_sb = consts.tile([4, 40], DT.float32, tag="blk4")
            nc.sync.dma_start(out=wencA[:], in_=wenc[0:128, :])
            nc.sync.dma_start(out=wencB[:], in_=wenc[128:181, :])
            nc.sync.dma_start(out=wdecA[:], in_=wdec[0:128, :])
            nc.sync.dma_start(out=wdecB[:], in_=wdec[128:180, :])
            nc.sync.dma_start(out=w3a_sb[:], in_=w3a[:, :])
            nc.sync.dma_start(out=w3b_sb[:], in_=w3b[:, :])
            nc.sync.dma_start(out=bout_sb[:], in_=bout[:, :])
            nc.sync.dma_start(out=ident_sb[:], in_=ident[:, :])
            nc.sync.dma_start(out=onescol_sb[:], in_=onescol[:, :])
            nc.sync.dma_start(out=onesrow_sb[:], in_=onesrow[:, :])
            nc.sync.dma_start(out=blk40_sb[:], in_=blk40[:, :])
            nc.sync.dma_start(out=blk4_sb[:], in_=blk4[:, :])

            xt_flat = xt[:, :, :].rearrange("f t b -> f (t b)")

            # persistent logits accumulator (one PSUM bank, whole kernel)
            acc = psum_pool.tile([40, BL], DT.float32, tag="acc", bufs=1)

            xslabs = [None] * (NCHUNK // SL)
            # cross-iteration state: chunk c's normalized E (decoder input)
            prev_en = None   # (EnA, EnB) of chunk it-1
            prev_dgt = None  # decoder gate tanh tiles of chunk it-1

            # Software-pipelined main loop: iteration `it` runs the encoder
            # of chunk `it` and the decoder of chunk `it-1`, so the softmax
            # normalizer chain (PE colsum -> DVE recip -> GPSIMD broadcast
            # -> DVE mult) has a full iteration of slack and the PE never
            # idles long enough for the HAM to re-throttle the clock.
            for it in range(NCHUNK + 1):
                c = it            # encoder chunk
                d = it - 1        # decoder chunk

                # ---- PE slot 1: encoder gate matmuls (chunk c) ----
                egps = []
                if c < NCHUNK:
                    if c % SL == 0:
                        xtA = xa_pool.tile([128, SL * CW], DT.bfloat16,
                                           tag="xtA")
                        xtB = xa_pool.tile([53, SL * CW], DT.bfloat16,
                                           tag="xtB")
                        nc.sync.dma_start(
                            out=xtA[:],
                            in_=xt_flat[0:128, c * CW:(c + SL) * CW])
                        nc.sync.dma_start(
                            out=xtB[:],
                            in_=xt_flat[128:181, c * CW:(c + SL) * CW])
                        xslabs[c // SL] = (xtA, xtB)
                    xtA, xtB = xslabs[c // SL]
                    co = (c % SL) * CW
                    rhsA = xtA[:, co:co + CW]
                    rhsB = xtB[:, co:co + CW]
                    for gi, (g0, gw) in enumerate(GRPS):
                        tag = "egA" if gw == 128 else "gB"
                        ps = psum_pool.tile([gw, CW], DT.float32, tag=tag,
                                            bufs=2)
                        nc.tensor.matmul(ps[:], wencA[:, g0:g0 + gw], rhsA,
                                         start=True, stop=False)
                        nc.tensor.matmul(ps[:], wencB[:, g0:g0 + gw], rhsB,
                                         start=False, stop=True)
                        egps.append(ps)

                # ---- PE slot 2: decoder gate matmuls (chunk d) ----
                dgps = []
                if d >= 0:
                    EnA_d, EnB_d = prev_en
                    for gi, (g0, gw) in enumerate(GRPS):
                        tag = "dgA" if gw == 128 else "gB"
                        ps = psum_pool.tile([gw, CW], DT.float32, tag=tag,
                                            bufs=2)
                        nc.tensor.matmul(ps[:], wdecA[:, g0:g0 + gw],
                                         EnA_d[:], start=True, stop=False)
                        nc.tensor.matmul(ps[:], wdecB[:, g0:g0 + gw],
                                         EnB_d[:], start=False, stop=True)
                        dgps.append(ps)

                # ---- ACT slot 1: encoder gate tanh (chunk c) ----
                egt = []
                if c < NCHUNK:
                    for gi, (g0, gw) in enumerate(GRPS):
                        tg = gt_pool.tile([gw, CW], DT.bfloat16,
                                          tag=f"etg{gi}")
                        nc.scalar.activation(tg[:], egps[gi][:], AF.Tanh)
                        egt.append(tg)

                # ---- ACT slot 2: decoder gate tanh (chunk d) ----
                dgt = []
                if d >= 0:
                    for gi, (g0, gw) in enumerate(GRPS):
                        tg = gt_pool.tile([gw, CW], DT.bfloat16,
                                          tag=f"dtg{gi}")
                        nc.scalar.activation(tg[:], dgps[gi][:], AF.Tanh)
                        dgt.append(tg)

                # ---- DVE/ACT: decoder elementwise (chunk d) ----
                if d >= 0:
                    wdA = mid_pool.tile([128, CW], DT.bfloat16, tag="wdA")
                    wdB = mid_pool.tile([HB, CW], DT.bfloat16, tag="wdB")
                    nc.vector.scalar_tensor_tensor(
                        wdA[:], dgt[0][:], 1.0, dgt[1][:], ALU.add, ALU.mult)
                    nc.vector.scalar_tensor_tensor(
                        wdB[:], dgt[3][:], 1.0, dgt[4][:], ALU.add, ALU.mult)
                    tdA = mid_pool.tile([128, CW], DT.bfloat16, tag="tdA")
                    tdB = mid_pool.tile([HB, CW], DT.bfloat16, tag="tdB")
                    nc.scalar.activation(tdA[:], wdA[:], AF.Tanh, scale=0.5)
                    nc.scalar.activation(tdB[:], wdB[:], AF.Tanh, scale=0.5)
                    # h2_dec; the 0.5 to get h is folded into W_out
                    hdA = hd_pool.tile([128, CW], DT.bfloat16, tag="hdA")
                    hdB = hd_pool.tile([HB, CW], DT.bfloat16, tag="hdB")
                    nc.vector.scalar_tensor_tensor(
                        hdA[:], dgt[2][:], 1.0, tdA[:], ALU.add, ALU.mult)
                    nc.vector.scalar_tensor_tensor(
                        hdB[:], dgt[5][:], 1.0, tdB[:], ALU.add, ALU.mult)

                    # ---- PE slot 3: output linear accumulation (chunk d) ----
                    for tt in range(TPC):
                        t = d * TPC + tt
                        nc.tensor.matmul(
                            acc[:], w3a_sb[:, t * 40:(t + 1) * 40],
                            hdA[:, tt * BL:(tt + 1) * BL],
                            start=(t == 0), stop=False)
                        nc.tensor.matmul(
                            acc[:], w3b_sb[:, t * 40:(t + 1) * 40],
                            hdB[:, tt * BL:(tt + 1) * BL],
                            start=False, stop=(t == T - 1))

                # ---- encoder elementwise + softmax normalizer (chunk c) ----
                if c < NCHUNK:
                    # w2c = (tanh(i/2)+1)*tanh(g) = 2*c ; tc = tanh(c)
                    # h2  = (tanh(o/2)+1)*tc = 2*h ; E = exp(h2/2) = exp(h)
                    w2cA = mid_pool.tile([128, CW], DT.bfloat16, tag="w2cA")
                    w2cB = mid_pool.tile([HB, CW], DT.bfloat16, tag="w2cB")
                    nc.vector.scalar_tensor_tensor(
                        w2cA[:], egt[0][:], 1.0, egt[1][:], ALU.add, ALU.mult)
                    nc.vector.scalar_tensor_tensor(
                        w2cB[:], egt[3][:], 1.0, egt[4][:], ALU.add, ALU.mult)
                    tcA = mid_pool.tile([128, CW], DT.bfloat16, tag="tcA")
                    tcB = mid_pool.tile([HB, CW], DT.bfloat16, tag="tcB")
                    nc.scalar.activation(tcA[:], w2cA[:], AF.Tanh, scale=0.5)
                    nc.scalar.activation(tcB[:], w2cB[:], AF.Tanh, scale=0.5)
                    h2A = mid_pool.tile([128, CW], DT.bfloat16, tag="h2A")
                    h2B = mid_pool.tile([HB, CW], DT.bfloat16, tag="h2B")
                    nc.vector.scalar_tensor_tensor(
                        h2A[:], egt[2][:], 1.0, tcA[:], ALU.add, ALU.mult)
                    nc.vector.scalar_tensor_tensor(
                        h2B[:], egt[5][:], 1.0, tcB[:], ALU.add, ALU.mult)
                    EA = mid_pool.tile([128, CW], DT.bfloat16, tag="EA")
                    EB = mid_pool.tile([HB, CW], DT.bfloat16, tag="EB")
                    nc.scalar.activation(EA[:], h2A[:], AF.Exp, scale=0.5)
                    nc.scalar.activation(EB[:], h2B[:], AF.Exp, scale=0.5)

                    # ---- PE slot 4: column sum of E ----
                    smp = psum_pool.tile([128, CW], DT.float32, tag="smp",
                                         bufs=1)
                    nc.tensor.matmul(smp[0:1, :], onescol_sb[:, 0:1], EA[:],
                                     start=True, stop=False)
                    nc.tensor.matmul(smp[0:1, :], onescol_sb[0:HB, 0:1],
                                     EB[:], start=False, stop=True)
                    rbf = sm_pool.tile([1, CW], DT.float32, tag="rbf")
                    nc.vector.reciprocal_approx_fast(rbf[:], smp[0:1, :])
                    rb = sm_pool.tile([1, CW], DT.bfloat16, tag="rb")
                    with nc.allow_low_precision(reason="softmax recip bf16"):
                        nc.vector.tensor_copy(rb[:], rbf[:])
                    rbc = sm_pool.tile([128, CW], DT.bfloat16, tag="rbc")
                    nc.gpsimd.partition_broadcast(rbc[:], rb[:])
                    EnA = en_pool.tile([128, CW], DT.bfloat16, tag="EnA")
                    EnB = en_pool.tile([HB, CW], DT.bfloat16, tag="EnB")
                    nc.vector.tensor_mul(EnA[:], EA[:], rbc[:])
                    nc.vector.tensor_mul(EnB[:], EB[:], rbc[0:HB, :])
                    prev_en = (EnA, EnB)

            # ---- end stage: bias, 4x10 group softmax, transpose, store ----
            lg = fin_pool.tile([40, BL], DT.float32, tag="lg")
            nc.vector.tensor_scalar(lg[:], acc[:], bout_sb[:, 0:1], None,
                                    ALU.add)
            eo = fin_pool.tile([40, BL], DT.float32, tag="eo")
            nc.scalar.activation(eo[:], lg[:], AF.Exp)
            ep = psum_pool.tile([128, CW], DT.float32, tag="smp", bufs=1)
            # group sums: [4, 256] = blk40^T(40x4) @ eo  (fp32 matmul)
            nc.tensor.matmul(ep[0:4, 0:BL], blk40_sb[:], eo[:],
                             start=True, stop=True)
            r4 = fin_pool.tile([4, BL], DT.float32, tag="r4")
            nc.vector.reciprocal(r4[:], ep[0:4, 0:BL])
            # broadcast r4 back to 40 partitions: blk4^T(4x40) @ r4
            nc.tensor.matmul(ep[0:40, BL:2 * BL], blk4_sb[:], r4[:],
                             start=True, stop=True)
            ob = fin_pool.tile([40, BL], DT.float32, tag="ob")
            nc.vector.tensor_tensor(ob[:], eo[:], ep[0:40, BL:2 * BL],
                                    ALU.mult)
            # transpose [40, 256] -> [256, 40] in two PE transposes,
            # reusing the smp psum bank (all prior reads complete by then)
            nc.tensor.transpose(ep[:, 0:40], ob[:, 0:128],
                                ident_sb[0:40, 0:40])
            nc.tensor.transpose(ep[:, 40:80], ob[:, 128:256],
                                ident_sb[0:40, 0:40])
            ot1 = fin_pool.tile([128, 40], DT.float32, tag="ot1")
            ot2 = fin_pool.tile([128, 40], DT.float32, tag="ot2")
            nc.scalar.copy(ot1[:], ep[:, 0:40])
            nc.scalar.copy(ot2[:], ep[:, 40:80])
            nc.sync.dma_start(out=out[0:128, :], in_=ot1[:])
            nc.sync.dma_start(out=out[128:256, :], in_=ot2[:])

    nc.finalize()
    return nc


def _get_program():
    global _PROGRAM
    if _PROGRAM is None:
        _PROGRAM = _build_program()
    return _PROGRAM


def _prep_lstm_weights(Wih, bih, bhh):
    W = np.asarray(Wih, np.float32)
    b = np.asarray(bih, np.float32) + np.asarray(bhh, np.float32)
    # torch gate order i, f, g, o; f unused (zero state). Halve i/o for
    # the tanh half-angle sigmoid identity.
    Wp = np.concatenate([0.5 * W[0:H], W[2 * H:3 * H], 0.5 * W[3 * H:4 * H]], 0)
    bp = np.concatenate([0.5 * b[0:H], b[2 * H:3 * H], 0.5 * b[3 * H:4 * H]], 0)
    return Wp, bp  # [540, 180], [540]


# permutation of the 540 (i,g,o)-rows into the on-chip column layout
_PERM = np.concatenate([
    np.arange(0, 128),          # iA
    np.arange(180, 308),        # gA
    np.arange(360, 488),        # oA
    np.arange(128, 180),        # iB
    np.arange(308, 360),        # gB
    np.arange(488, 540),        # oB
])


def kernel(x, W_ih_enc, b_ih_enc, b_hh_enc, W_ih_dec, b_ih_dec, b_hh_dec,
           W_out, b_out):
    global LAST_RESULTS
    x = np.asarray(x)
    B = x.shape[0]
    assert B == B_FULL, f"kernel hardcoded for B={B_FULL}, got {B}"

    # x[b, c, s, t] with feature f = c*60+s -> per-core xt[f, t, b];
    # row f=180 of ones provides the encoder bias via the augmented
    # contraction dim.
    xr = x.reshape(B, H, T)
    xts = []
    for c in range(NCORES):
        xt = np.empty((181, T, BL), BF16)
        xt[:180] = xr[c * BL:(c + 1) * BL].transpose(1, 2, 0)
        xt[180] = 1.0
        xts.append(xt)

    We, be = _prep_lstm_weights(W_ih_enc, b_ih_enc, b_hh_enc)
    wenc = np.concatenate([We.T, be[None, :]], 0)[:, _PERM]
    wenc = np.ascontiguousarray(wenc).astype(BF16)  # [181, 540]

    Wd, bd = _prep_lstm_weights(W_ih_dec, b_ih_dec, b_hh_dec)
    # softmax rows sum to 1 -> bias folds into every column of Wdec
    wdec = (Wd.T + bd[None, :])[:, _PERM]
    wdec = np.ascontiguousarray(wdec).astype(BF16)  # [180, 540]

    # logits use h = h2/2 -> fold the 0.5 into W_out; W3[h, t, j]
    W3 = (0.5 * np.asarray(W_out, np.float32)).reshape(40, T, H)
    W3 = np.ascontiguousarray(W3.transpose(2, 1, 0))  # [180, 240, 40]
    w3a = np.ascontiguousarray(W3[0:128]).reshape(128, T * 40).astype(BF16)
    w3b = np.ascontiguousarray(W3[128:180]).reshape(HB, T * 40).astype(BF16)

    bout = np.asarray(b_out, np.float32).reshape(40, 1)
    ident = np.eye(64, dtype=np.float32)
    onescol = np.ones((128, 1), BF16)
    onesrow = np.ones((1, 128), BF16)
    gidx = np.arange(40) // 10
    blk40 = (gidx[:, None] == np.arange(4)[None, :]).astype(np.float32)
    blk4 = np.ascontiguousarray(blk40.T)

    nc = _get_program()
    in_maps = []
    for c in range(NCORES):
        in_maps.append({
            "xt": xts[c],
            "wenc": wenc,
            "wdec": wdec,
            "w3a": w3a,
            "w3b": w3b,
            "bout": bout,
            "ident": ident,
            "onescol": onescol,
            "onesrow": onesrow,
            "blk40": blk40,
            "blk4": blk4,
        })
    trace = bool(int(os.environ.get("KERNEL_TRACE", "0")))
    res = bass_utils.run_bass_kernel_spmd(
        nc, in_maps, core_ids=list(range(NCORES)), trace=trace)
    LAST_RESULTS = res
    out = np.concatenate([r["out"] for r in res.results], 0)  # [B, 40]
    return out.reshape(B, 4, 10).astype(np.float32)


# revision 19
# speedup vs baseline: 2.0070x; 1.1580x over previous
"""Trainium2 Bass kernel for nn_Net_91122026151953.

Net (per batch row b):
  xe = x.transpose(0,3,1,2).reshape(B, 240, 180)            # [B,T,180]
  h_enc = lstm_cell_zero_state(xe, Wenc, b)                 # sigmoid/tanh gates, no recurrence
  enc   = softmax(h_enc, axis=-1)
  h_dec = lstm_cell_zero_state(enc, Wdec, b)
  out   = softmax((h_dec.reshape(B,T*180) @ W_out.T + b_out).reshape(B,4,10), -1)

Strategy: pure data-parallel over 8 cores (256 rows each).  "Orientation
B": features/hidden on SBUF partitions, (t, b) on the free dim with all
256 batch rows contiguous (t-outer).  This removes every transpose (the
encoder input arrives feature-major from a host-side transpose; the
decoder input E^T and the final-linear input h_dec^T are produced
directly in the needed layout), and lets the output Linear run as 4
N=256 matmuls per chunk accumulating into one persistent [40,256] PSUM
tile instead of thousands of N=32 matmuls.

All transcendentals use only tanh/exp (sigmoid(x) = 0.5*(1+tanh(x/2)),
halves folded into weights) so a single ACT table set is used.  The
decoder bias is folded into Wdec columns (softmax rows sum to 1); the
encoder softmax normalizer is applied to E explicitly (PE ones-matmul
column sum -> reciprocal -> PE broadcast -> multiply).
"""

import os
import numpy as np
import ml_dtypes

import concourse.bass as bass
import concourse.tile as tile
from concourse import bacc, mybir
from concourse import bass_utils

BF16 = ml_dtypes.bfloat16
FP32 = np.float32

H = 180           # hidden
T = 240           # timesteps
G3 = 540          # 3 used gates (i, g, o)
NCORES = 8
B_FULL = 2048
BL = B_FULL // NCORES   # rows per core = 256
CW = 512                # chunk width (free columns) = 2 t-steps * 256 b
TPC = CW // BL          # t-steps per chunk = 2
NCHUNK = T // TPC       # 120 chunks
SL = 4                  # chunks per x-load slab

# gate column layout in the permuted 540-wide weight matrices
#   iA: 0:128    gA: 128:256   oA: 256:384   (h = 0..127)
#   iB: 384:436  gB: 436:488   oB: 488:540   (h = 128..179)
GRPS = [(0, 128), (128, 128), (256, 128), (384, 52), (436, 52), (488, 52)]
HB = 52           # second h block size
G3P = 544         # G3 padded so the DoubleRow lhsT outer stride is 16B-aligned

AF = mybir.ActivationFunctionType
ALU = mybir.AluOpType
DT = mybir.dt
FP8 = ml_dtypes.float8_e4m3

# fp8 dynamic-range scales (folded back out via activation scale)
WENC_SCALE = 64.0   # gate weights are ~uniform(+-0.075); x is ~N(0,1)
WDEC_SCALE = 8.0
EN_SCALE = 32.0     # normalized E entries are ~1/180

_PROGRAM = None
LAST_RESULTS = None


def _build_program():
    nc = bacc.Bacc(None, name="lstm_net_b")

    # x and gate weights ship as fp8 e4m3 in DoubleRow layout: dim1 is the
    # two 128-row k-subtiles (features 0:128 and 128:181 zero-padded), so
    # each gate group's full k=181 contraction is ONE DoubleRow matmul.
    xt = nc.dram_tensor("xt", [128, 2, T, BL], DT.float8e4,
                        kind="ExternalInput")
    wenc = nc.dram_tensor("wenc", [128, 2, G3P], DT.float8e4,
                          kind="ExternalInput")
    wdec = nc.dram_tensor("wdec", [128, 2, G3P], DT.float8e4,
                          kind="ExternalInput")
    w3a = nc.dram_tensor("w3a", [128, T * 40], DT.bfloat16, kind="ExternalInput")
    w3b = nc.dram_tensor("w3b", [HB, T * 40], DT.bfloat16, kind="ExternalInput")
    bout = nc.dram_tensor("bout", [40, 1], DT.float32, kind="ExternalInput")
    ident = nc.dram_tensor("ident", [64, 64], DT.float32, kind="ExternalInput")
    onescol = nc.dram_tensor("onescol", [128, 1], DT.bfloat16, kind="ExternalInput")
    onesrow = nc.dram_tensor("onesrow", [1, 128], DT.bfloat16, kind="ExternalInput")
    blk40 = nc.dram_tensor("blk40", [40, 4], DT.float32, kind="ExternalInput")
    blk4 = nc.dram_tensor("blk4", [4, 40], DT.float32, kind="ExternalInput")
    out = nc.dram_tensor("out", [BL, 40], DT.float32, kind="ExternalOutput")

    with tile.TileContext(nc) as tc:
        with (
            tc.tile_pool(name="consts", bufs=1) as consts,
            tc.tile_pool(name="xa", bufs=2) as xa_pool,
            tc.tile_pool(name="gt", bufs=2) as gt_pool,       # gate tanh outputs
            tc.tile_pool(name="mid", bufs=2) as mid_pool,     # w2c/tc/h2/E
            tc.tile_pool(name="en", bufs=2) as en_pool,       # normalized E
            tc.tile_pool(name="hd", bufs=2) as hd_pool,       # decoder h2
            tc.tile_pool(name="sm", bufs=2) as sm_pool,       # small softmax bits
            tc.tile_pool(name="fin", bufs=1) as fin_pool,
            tc.tile_pool(name="psum", bufs=1, space="PSUM") as psum_pool,
        ):
            # ---- constants ----
            wenc_sb = consts.tile([128, 2, G3P], DT.float8e4, tag="wenc_sb")
            wdec_sb = consts.tile([128, 2, G3P], DT.float8e4, tag="wdec_sb")
            w3a_sb = consts.tile([128, T * 40], DT.bfloat16, tag="w3a")
            w3b_sb = consts.tile([HB, T * 40], DT.bfloat16, tag="w3b")
            bout_sb = consts.tile([40, 1], DT.float32, tag="bout")
            ident_sb = consts.tile([64, 64], DT.float32, tag="ident")
            onescol_sb = consts.tile([128, 1], DT.bfloat16, tag="onescol")
            onesrow_sb = consts.tile([1, 128], DT.bfloat16, tag="onesrow")
            blk40_sb = consts.tile([40, 4], DT.float32, tag="blk40")
            blk4_sb = consts.tile([4, 40], DT.float32, tag="blk4")
            nc.sync.dma_start(out=wenc_sb[:], in_=wenc[:, :, :])
            nc.sync.dma_start(out=wdec_sb[:], in_=wdec[:, :, :])
            nc.sync.dma_start(out=w3a_sb[:], in_=w3a[:, :])
            nc.sync.dma_start(out=w3b_sb[:], in_=w3b[:, :])
            nc.sync.dma_start(out=bout_sb[:], in_=bout[:, :])
            nc.sync.dma_start(out=ident_sb[:], in_=ident[:, :])
            nc.sync.dma_start(out=onescol_sb[:], in_=onescol[:, :])
            nc.sync.dma_start(out=onesrow_sb[:], in_=onesrow[:, :])
            nc.sync.dma_start(out=blk40_sb[:], in_=blk40[:, :])
            nc.sync.dma_start(out=blk4_sb[:], in_=blk4[:, :])

            xt_flat = xt[:, :, :, :].rearrange("f s t b -> f s (t b)")

            # persistent logits accumulator (one PSUM bank, whole kernel)
            acc = psum_pool.tile([40, BL], DT.float32, tag="acc", bufs=1)

            # decoder rhs in DoubleRow layout: [128, 2, CW] fp8; subtile 1
            # rows 52:128 stay zero (matching the zero weight rows). Two
            # manually-alternated buffers, zeroed once up front.
            en0 = en_pool.tile([128, 2, CW], DT.float8e4, tag="En", bufs=2)
            en1 = en_pool.tile([128, 2, CW], DT.float8e4, tag="En", bufs=2)
            nc.vector.memset(en0[:], 0.0)
            nc.vector.memset(en1[:], 0.0)
            en_tiles = [en0, en1]

            xslabs = [None] * (NCHUNK // SL)
            # cross-iteration state: chunk c's normalized E (decoder input)
            prev_en = None   # En tile of chunk it-1

            # Software-pipelined main loop: iteration `it` runs the encoder
            # of chunk `it` and the decoder of chunk `it-1`, so the softmax
            # normalizer chain (PE colsum -> DVE recip -> GPSIMD broadcast
            # -> DVE mult) has a full iteration of slack and the PE never
            # idles long enough for the HAM to re-throttle the clock.
            for it in range(NCHUNK + 1):
                c = it            # encoder chunk
                d = it - 1        # decoder chunk

                # ---- PE slot 1: encoder gate matmuls (chunk c) ----
                egps = []
                if c < NCHUNK:
                    if c % SL == 0:
                        xtA = xa_pool.tile([128, 2, SL * CW], DT.float8e4,
                                           tag="xtA")
                        nc.sync.dma_start(
                            out=xtA[:],
                            in_=xt_flat[:, :, c * CW:(c + SL) * CW])
                        xslabs[c // SL] = xtA
                    xtA = xslabs[c // SL]
                    co = (c % SL) * CW
                    rhs = xtA[:, :, co:co + CW]
                    for gi, (g0, gw) in enumerate(GRPS):
                        tag = "egA" if gw == 128 else "gB"
                        ps = psum_pool.tile([gw, CW], DT.float32, tag=tag,
                                            bufs=2)
                        nc.tensor.matmul(
                            ps[:], wenc_sb[:, :, g0:g0 + gw], rhs,
                            start=True, stop=True,
                            perf_mode=mybir.MatmulPerfMode.DoubleRow)
                        egps.append(ps)

                # ---- PE slot 2: decoder gate matmuls (chunk d) ----
                dgps = []
                if d >= 0:
                    en_d = prev_en
                    for gi, (g0, gw) in enumerate(GRPS):
                        tag = "dgA" if gw == 128 else "gB"
                        ps = psum_pool.tile([gw, CW], DT.float32, tag=tag,
                                            bufs=2)
                        nc.tensor.matmul(
                            ps[:], wdec_sb[:, :, g0:g0 + gw], en_d[:],
                            start=True, stop=True,
                            perf_mode=mybir.MatmulPerfMode.DoubleRow)
                        dgps.append(ps)

                # ---- ACT slot 1: encoder gate tanh (chunk c) ----
                egt = []
                if c < NCHUNK:
                    for gi, (g0, gw) in enumerate(GRPS):
                        tg = gt_pool.tile([gw, CW], DT.bfloat16,
                                          tag=f"etg{gi}")
                        nc.scalar.activation(tg[:], egps[gi][:], AF.Tanh,
                                             scale=1.0 / WENC_SCALE)
                        egt.append(tg)

                # ---- ACT slot 2: decoder gate tanh (chunk d) ----
                dgt = []
                if d >= 0:
                    for gi, (g0, gw) in enumerate(GRPS):
                        tg = gt_pool.tile([gw, CW], DT.bfloat16,
                                          tag=f"dtg{gi}")
                        nc.scalar.activation(
                            tg[:], dgps[gi][:], AF.Tanh,
                            scale=1.0 / (WDEC_SCALE * EN_SCALE))
                        dgt.append(tg)

                # ---- DVE/ACT: decoder elementwise (chunk d) ----
                if d >= 0:
                    wdA = mid_pool.tile([128, CW], DT.bfloat16, tag="wdA")
                    wdB = mid_pool.tile([HB, CW], DT.bfloat16, tag="wdB")
                    nc.vector.scalar_tensor_tensor(
                        wdA[:], dgt[0][:], 1.0, dgt[1][:], ALU.add, ALU.mult)
                    nc.vector.scalar_tensor_tensor(
                        wdB[:], dgt[3][:], 1.0, dgt[4][:], ALU.add, ALU.mult)
                    tdA = mid_pool.tile([128, CW], DT.bfloat16, tag="tdA")
                    tdB = mid_pool.tile([HB, CW], DT.bfloat16, tag="tdB")
                    nc.scalar.activation(tdA[:], wdA[:], AF.Tanh, scale=0.5)
                    nc.scalar.activation(tdB[:], wdB[:], AF.Tanh, scale=0.5)
                    # h2_dec; the 0.5 to get h is folded into W_out
                    hdA = hd_pool.tile([128, CW], DT.bfloat16, tag="hdA")
                    hdB = hd_pool.tile([HB, CW], DT.bfloat16, tag="hdB")
                    nc.vector.scalar_tensor_tensor(
                        hdA[:], dgt[2][:], 1.0, tdA[:], ALU.add, ALU.mult)
                    nc.vector.scalar_tensor_tensor(
                        hdB[:], dgt[5][:], 1.0, tdB[:], ALU.add, ALU.mult)

                    # ---- PE slot 3: output linear accumulation (chunk d) ----
                    for tt in range(TPC):
                        t = d * TPC + tt
                        nc.tensor.matmul(
                            acc[:], w3a_sb[:, t * 40:(t + 1) * 40],
                            hdA[:, tt * BL:(tt + 1) * BL],
                            start=(t == 0), stop=False)
                        nc.tensor.matmul(
                            acc[:], w3b_sb[:, t * 40:(t + 1) * 40],
                            hdB[:, tt * BL:(tt + 1) * BL],
                            start=False, stop=(t == T - 1))

                # ---- encoder elementwise + softmax normalizer (chunk c) ----
                if c < NCHUNK:
                    # w2c = (tanh(i/2)+1)*tanh(g) = 2*c ; tc = tanh(c)
                    # h2  = (tanh(o/2)+1)*tc = 2*h ; E = exp(h2/2) = exp(h)
                    w2cA = mid_pool.tile([128, CW], DT.bfloat16, tag="w2cA")
                    w2cB = mid_pool.tile([HB, CW], DT.bfloat16, tag="w2cB")
                    nc.vector.scalar_tensor_tensor(
                        w2cA[:], egt[0][:], 1.0, egt[1][:], ALU.add, ALU.mult)
                    nc.vector.scalar_tensor_tensor(
                        w2cB[:], egt[3][:], 1.0, egt[4][:], ALU.add, ALU.mult)
                    tcA = mid_pool.tile([128, CW], DT.bfloat16, tag="tcA")
                    tcB = mid_pool.tile([HB, CW], DT.bfloat16, tag="tcB")
                    nc.scalar.activation(tcA[:], w2cA[:], AF.Tanh, scale=0.5)
                    nc.scalar.activation(tcB[:], w2cB[:], AF.Tanh, scale=0.5)
                    h2A = mid_pool.tile([128, CW], DT.bfloat16, tag="h2A")
                    h2B = mid_pool.tile([HB, CW], DT.bfloat16, tag="h2B")
                    nc.vector.scalar_tensor_tensor(
                        h2A[:], egt[2][:], 1.0, tcA[:], ALU.add, ALU.mult)
                    nc.vector.scalar_tensor_tensor(
                        h2B[:], egt[5][:], 1.0, tcB[:], ALU.add, ALU.mult)
                    EA = mid_pool.tile([128, CW], DT.bfloat16, tag="EA")
                    EB = mid_pool.tile([HB, CW], DT.bfloat16, tag="EB")
                    nc.scalar.activation(EA[:], h2A[:], AF.Exp, scale=0.5)
                    nc.scalar.activation(EB[:], h2B[:], AF.Exp, scale=0.5)

                    # ---- PE slot 4: column sum of E ----
                    smp = psum_pool.tile([128, CW], DT.float32, tag="smp",
                                         bufs=1)
                    nc.tensor.matmul(smp[0:1, :], onescol_sb[:, 0:1], EA[:],
                                     start=True, stop=False)
                    nc.tensor.matmul(smp[0:1, :], onescol_sb[0:HB, 0:1],
                                     EB[:], start=False, stop=True)
                    rbf = sm_pool.tile([1, CW], DT.float32, tag="rbf")
                    nc.vector.reciprocal_approx_fast(rbf[:], smp[0:1, :])
                    rb = sm_pool.tile([1, CW], DT.bfloat16, tag="rb")
                    with nc.allow_low_precision(reason="softmax recip bf16"):
                        # fold the fp8 dynamic-range scale into 1/s
                        nc.vector.tensor_scalar(rb[:], rbf[:],
                                                float(EN_SCALE), None,
                                                ALU.mult)
                    rbc = sm_pool.tile([128, CW], DT.bfloat16, tag="rbc")
                    nc.gpsimd.partition_broadcast(rbc[:], rb[:])
                    en_t = en_tiles[c % 2]
                    with nc.allow_low_precision(reason="decoder rhs fp8"):
                        nc.vector.tensor_mul(en_t[:, 0, :], EA[:], rbc[:])
                        nc.vector.tensor_mul(en_t[0:HB, 1, :], EB[:],
                                             rbc[0:HB, :])
                    prev_en = en_t

            # ---- end stage: bias, 4x10 group softmax, transpose, store ----
            lg = fin_pool.tile([40, BL], DT.float32, tag="lg")
            nc.vector.tensor_scalar(lg[:], acc[:], bout_sb[:, 0:1], None,
                                    ALU.add)
            eo = fin_pool.tile([40, BL], DT.float32, tag="eo")
            nc.scalar.activation(eo[:], lg[:], AF.Exp)
            ep = psum_pool.tile([128, CW], DT.float32, tag="smp", bufs=1)
            # group sums: [4, 256] = blk40^T(40x4) @ eo  (fp32 matmul)
            nc.tensor.matmul(ep[0:4, 0:BL], blk40_sb[:], eo[:],
                             start=True, stop=True)
            r4 = fin_pool.tile([4, BL], DT.float32, tag="r4")
            nc.vector.reciprocal(r4[:], ep[0:4, 0:BL])
            # broadcast r4 back to 40 partitions: blk4^T(4x40) @ r4
            nc.tensor.matmul(ep[0:40, BL:2 * BL], blk4_sb[:], r4[:],
                             start=True, stop=True)
            ob = fin_pool.tile([40, BL], DT.float32, tag="ob")
            nc.vector.tensor_tensor(ob[:], eo[:], ep[0:40, BL:2 * BL],
                                    ALU.mult)
            # transpose [40, 256] -> [256, 40] in two PE transposes,
            # reusing the smp psum bank (all prior reads complete by then)
            nc.tensor.transpose(ep[:, 0:40], ob[:, 0:128],
                                ident_sb[0:40, 0:40])
            nc.tensor.transpose(ep[:, 40:80], ob[:, 128:256],
                                ident_sb[0:40, 0:40])
            ot1 = fin_pool.tile([128, 40], DT.float32, tag="ot1")
            ot2 = fin_pool.tile([128, 40], DT.float32, tag="ot2")
            nc.scalar.copy(ot1[:], ep[:, 0:40])
            nc.scalar.copy(ot2[:], ep[:, 40:80])
            nc.sync.dma_start(out=out[0:128, :], in_=ot1[:])
            nc.sync.dma_start(out=out[128:256, :], in_=ot2[:])

    nc.finalize()
    return nc


def _get_program():
    global _PROGRAM
    if _PROGRAM is None:
        _PROGRAM = _build_program()
    return _PROGRAM


def _prep_lstm_weights(Wih, bih, bhh):
    W = np.asarray(Wih, np.float32)
    b = np.asarray(bih, np.float32) + np.asarray(bhh, np.float32)
    # torch gate order i, f, g, o; f unused (zero state). Halve i/o for
    # the tanh half-angle sigmoid identity.
    Wp = np.concatenate([0.5 * W[0:H], W[2 * H:3 * H], 0.5 * W[3 * H:4 * H]], 0)
    bp = np.concatenate([0.5 * b[0:H], b[2 * H:3 * H], 0.5 * b[3 * H:4 * H]], 0)
    return Wp, bp  # [540, 180], [540]


# permutation of the 540 (i,g,o)-rows into the on-chip column layout
_PERM = np.concatenate([
    np.arange(0, 128),          # iA
    np.arange(180, 308),        # gA
    np.arange(360, 488),        # oA
    np.arange(128, 180),        # iB
    np.arange(308, 360),        # gB
    np.arange(488, 540),        # oB
])


def kernel(x, W_ih_enc, b_ih_enc, b_hh_enc, W_ih_dec, b_ih_dec, b_hh_dec,
           W_out, b_out):
    global LAST_RESULTS
    x = np.asarray(x)
    B = x.shape[0]
    assert B == B_FULL, f"kernel hardcoded for B={B_FULL}, got {B}"

    # x[b, c, s, t] with feature f = c*60+s -> per-core xt[f-sub, 2, t, b]
    # in fp8 DoubleRow layout (k-subtiles 0:128 and 128:181 zero-padded);
    # row f=180 of ones provides the encoder bias via the augmented
    # contraction dim.
    xr = x.reshape(B, H, T)
    xts = []
    for c in range(NCORES):
        xt = np.zeros((128, 2, T, BL), FP8)
        xc = xr[c * BL:(c + 1) * BL].transpose(1, 2, 0)  # [180, T, BL]
        xt[:, 0] = xc[0:128]
        xt[0:52, 1] = xc[128:180]
        xt[52, 1] = 1.0
        xts.append(xt)

    We, be = _prep_lstm_weights(W_ih_enc, b_ih_enc, b_hh_enc)
    wenc2 = np.concatenate([We.T, be[None, :]], 0)[:, _PERM] * WENC_SCALE
    wenc = np.zeros((128, 2, G3P), FP8)
    wenc[:, 0, 0:G3] = wenc2[0:128]
    wenc[0:53, 1, 0:G3] = wenc2[128:181]

    Wd, bd = _prep_lstm_weights(W_ih_dec, b_ih_dec, b_hh_dec)
    # softmax rows sum to 1 -> bias folds into every column of Wdec
    wdec2 = (Wd.T + bd[None, :])[:, _PERM] * WDEC_SCALE
    wdec = np.zeros((128, 2, G3P), FP8)
    wdec[:, 0, 0:G3] = wdec2[0:128]
    wdec[0:52, 1, 0:G3] = wdec2[128:180]

    # logits use h = h2/2 -> fold the 0.5 into W_out; W3[h, t, j]
    W3 = (0.5 * np.asarray(W_out, np.float32)).reshape(40, T, H)
    W3 = np.ascontiguousarray(W3.transpose(2, 1, 0))  # [180, 240, 40]
    w3a = np.ascontiguousarray(W3[0:128]).reshape(128, T * 40).astype(BF16)
    w3b = np.ascontiguousarray(W3[128:180]).reshape(HB, T * 40).astype(BF16)

    bout = np.asarray(b_out, np.float32).reshape(40, 1)
    ident = np.eye(64, dtype=np.float32)
    onescol = np.ones((128, 1), BF16)
    onesrow = np.ones((1, 128), BF16)
    gidx = np.arange(40) // 10
    blk40 = (gidx[:, None] == np.arange(4)[None, :]).astype(np.float32)
    blk4 = np.ascontiguousarray(blk40.T)

    nc = _get_program()
    in_maps = []
    for c in range(NCORES):
        in_maps.append({
            "xt": xts[c],
            "wenc": wenc,
            "wdec": wdec,
            "w3a": w3a,
            "w3b": w3b,
            "bout": bout,
            "ident": ident,
            "onescol": onescol,
            "onesrow": onesrow,
            "blk40": blk40,
            "blk4": blk4,
        })
    trace = bool(int(os.environ.get("KERNEL_TRACE", "0")))
    res = bass_utils.run_bass_kernel_spmd(
        nc, in_maps, core_ids=list(range(NCORES)), trace=trace)
    LAST_RESULTS = res
    out = np.concatenate([r["out"] for r in res.results], 0)  # [B, 40]
    return out.reshape(B, 4, 10).astype(np.float32)
